# revision 3
# baseline (speedup 1.0000x reference)
"""Trainium2 Bass kernel for nn_CompetitiveNetwork (competitive-binding solve).

Math (per batch row b):
    K  = clip(exp(K_raw), 0, 1e3)   BT = clip(exp(BT_raw), 0, 1e3)
    fixed point:  BF' = 1/(1 + K^T AF);  AF = AT / (1 + (K*diag(BT)) BF')
    readout:      BF' = 1/(1 + K^T AF);  Y = AF^T (K*W*BT) BF' + b

v2 strategy (vs the 21-plain-iteration v1):
  - Successive over-relaxation on the AF-side reciprocal state R~:
      R~_k = (1-w_k) R~_{k-1} + w_k * Newton(1/(1+T_k); seed=R~_{k-1})
    folded into ONE custom DVE op per tile by pre-scaling the T-matmul
    weights with w_k (PSUM holds w_k*T, newton1p immediates s0=w_k,
    s1=1+w_k).  With a ramped w schedule 10 iterations reach ~1.6e-3
    rel err vs the 21-iteration reference (validated in numpy replica;
    harness gate is 2e-2).
  - Block-diagonal 128x128 fp16 weights: both batch-substreams in one
    matmul (PE cost = moving rows only), halving PE time vs quadrant
    pairs.
  - First ACT_R_ITERS iterations compute both reciprocals on ACT
    (exact, seedless) - the Newton basin needs |seed*(1+T)| < 2 which
    early iterates violate.
  - AT shipped as fp16 from host (halves input DMA, removes casts).

Sharding: pure data-parallel over batch (16384 -> 8 cores x 2048).
Layout: features on partitions (2 streams of 64 stacked -> 128), batch
on free dim; 2 column chunks of 512 per core.
"""

import numpy as np

import concourse.bacc as bacc
import concourse.mybir as mybir
from concourse.tile import TileContext
from concourse.bass_utils import run_bass_kernel_spmd


# --- custom DVE op: NEWTON1P_ANT (one 4-stage DVE instruction) ---
# out = (c1 - (in0 + c0) * in1) * in1
# With in0 = w*T (w pre-scaled into the matmul weights), c0 = w,
# c1 = 1+w, in1 = seed s:  out = (1-w)*s + w*(2-(1+T)s)s  — a Newton
# refinement of 1/(1+T) blended with SOR weight w, in one op.

import concourse.dve_ops as dve_ops
from concourse.dve_ops import DveOp
from concourse.dve_spec import Spec, Src0, Src1, C0, C1, lower


def _ref_newton1p(in0, in1, c0, c1, c2):
    return ((c1 - (in0.astype(np.float32) + c0) * in1) * in1).astype(np.float32)


def _make_op(shas):
    return DveOp(
        "NEWTON1P_ANT",
        Spec(
            body=(C1 - (Src0 + C0) * Src1) * Src1,
            reference=_ref_newton1p,
        ),
        subdim=False,
        uops_sha=shas,
    )


def register():
    for op in dve_ops.OPS:
        if op.name == "NEWTON1P_ANT":
            return op
    probe = _make_op({})
    opcode = dve_ops._CUSTOM_DVE_ROW_BASE + len(dve_ops.OPS)
    shas = {}
    for ver in ("v3", "v4"):
        try:
            from concourse.dve_uop import DveOpSpec
            res = DveOpSpec(name=probe.name, opcode=opcode,
                            uops=lower(probe.spec, ver=ver),
                            rd1_en=True)
            shas[ver] = res.sha(ver)
        except Exception as e:
            print(f"lower {ver} failed: {e}")
    op = _make_op(shas)
    dve_ops.OPS.append(op)
    dve_ops.CUSTOM_DVE_SPECS[op.name] = op.spec
    dve_ops._SUB_OPCODE_FOR_NAME[op.name] = (
        dve_ops._CUSTOM_DVE_ROW_BASE + len(dve_ops.OPS) - 1)
    return op


def newton1p(nc_vector, out, in0, in1, s0=1.0, s1=2.0):
    """out = (s1 - (in0 + s0) * in1) * in1 on the DVE."""
    op = register()
    return nc_vector._custom_dve(op, out=out, in0=in0, in1=in1,
                                 s0=s0, s1=s1, imm2=0.0)


B, NA, NB = 16384, 64, 64
N_CORES = 8
B_CORE = B // N_CORES          # 2048 batch rows per core
N_CHUNK = 2
FD = B_CORE // 2 // N_CHUNK    # 512

N_ITERS = 10
ACT_R_ITERS = 3                # seedless ACT reciprocals until Newton basin
# SOR schedule: identity while bootstrapping, then ramp (numpy-tuned)
OMEGAS = [1.0] * ACT_R_ITERS + [
    round(1.15 + (1.40 - 1.15) * i / (N_ITERS - ACT_R_ITERS - 1), 4)
    for i in range(N_ITERS - ACT_R_ITERS)
]

FP32 = mybir.dt.float32
FP16 = mybir.dt.float16

_CACHE = {}


def _act_recip(nc, out_ap, in_ap, bias=1.0, scale=1.0):
    """out = 1 / (scale*in + bias) on the ACT engine."""
    eng = nc.scalar
    ins = [eng.lower_ap(in_ap),
           mybir.ImmediateValue(dtype=FP32, value=bias),
           mybir.ImmediateValue(dtype=FP32, value=scale),
           mybir.ImmediateValue(dtype=FP32, value=0.0)]
    eng.add_instruction(mybir.InstActivation(
        name=nc.get_next_instruction_name(),
        func=mybir.ActivationFunctionType.Reciprocal,
        ins=ins, outs=[eng.lower_ap(out_ap)]))


def _build_module(repeat=1):
    register()
    nc = bacc.Bacc()
    # weight stack: [w1, m2, w2_0..w2_{n-1}] as 128x128 fp16 block-diagonals
    NW = 2 + N_ITERS
    wstack = nc.dram_tensor("wstack", (128, 128 * NW), FP16, kind="ExternalInput")
    att = nc.dram_tensor("att", (128, N_CHUNK * FD), FP16, kind="ExternalInput")
    yout = nc.dram_tensor("yout", (2 * N_CHUNK, FD), FP32, kind="ExternalOutput")

    with TileContext(nc) as tc, \
         tc.tile_pool(name="const", bufs=1) as cpool, \
         tc.tile_pool(name="state", bufs=3) as spool, \
         tc.tile_pool(name="work", bufs=2) as wpool, \
         tc.tile_pool(name="psum", bufs=8, space="PSUM") as ppool:

        w1h = cpool.tile([128, 128], FP16, tag="w1h")
        m2h = cpool.tile([128, 128], FP16, tag="m2h")
        nc.sync.dma_start(out=w1h[:], in_=wstack[:, 0:128])
        nc.sync.dma_start(out=m2h[:], in_=wstack[:, 128:256])
        w2h = []
        for k in range(N_ITERS):
            t = cpool.tile([128, 128], FP16, tag=f"w2h{k}")
            nc.sync.dma_start(out=t[:], in_=wstack[:, 128 * (2 + k):128 * (3 + k)])
            w2h.append(t)
        at16 = cpool.tile([128, N_CHUNK * FD], FP16, tag="at16")
        nc.sync.dma_start(out=at16[:], in_=att[:, :])
        # ones [128, 2]: col0 = ones on partitions 0:64, col1 on 64:128
        # (one matmul reduces both streams: out row 0 = stream0, row1 = stream1)
        onesb = cpool.tile([128, 2], FP16, tag="onesb")
        nc.vector.memset(onesb[:], 0.0)
        nc.vector.memset(onesb[0:64, 0:1], 1.0)
        nc.vector.memset(onesb[64:128, 1:2], 1.0)

        def at_sl(c):
            return at16[:, c * FD:(c + 1) * FD]

        for _rep in range(repeat):
            af = [at_sl(0), at_sl(1)]
            bf = [None] * N_CHUNK
            rr = [None] * N_CHUNK

            def emit_halfstep(c, h):
                k = h // 2
                w = OMEGAS[k]
                if h % 2 == 0:
                    # S = K^T AF ; BF' = 1/(1+S)
                    ps = ppool.tile([128, FD], FP32, tag="ps")
                    nc.tensor.matmul(out=ps[:], lhsT=w1h[:], rhs=af[c],
                                     start=True, stop=True)
                    bf_n = spool.tile([128, FD], FP16, tag=f"bf{c}")
                    _act_recip(nc, bf_n[:], ps[:])
                    bf[c] = bf_n
                else:
                    # PSUM = w*T (w baked into weights); R~ update
                    ps2 = ppool.tile([128, FD], FP32, tag="ps")
                    nc.tensor.matmul(out=ps2[:], lhsT=w2h[k][:], rhs=bf[c][:],
                                     start=True, stop=True)
                    r_n = spool.tile([128, FD], FP16, tag=f"r{c}")
                    if k < ACT_R_ITERS:
                        _act_recip(nc, r_n[:], ps2[:], bias=1.0, scale=1.0 / w)
                    else:
                        newton1p(nc.vector, r_n[:], ps2[:], rr[c][:],
                                 s0=w, s1=1.0 + w)
                    rr[c] = r_n
                    af_n = spool.tile([128, FD], FP16, tag=f"af{c}")
                    if c == 0:
                        nc.vector.tensor_mul(af_n[:], at_sl(c), r_n[:])
                    else:
                        nc.gpsimd.tensor_mul(af_n[:], at_sl(c), r_n[:])
                    af[c] = af_n

            # chunk 1 one half-step behind chunk 0 for steady pipelining
            H = 2 * N_ITERS
            for t in range(H + 1):
                if t < H:
                    emit_halfstep(0, t)
                if t >= 1:
                    emit_halfstep(1, t - 1)

            # readout: BF_f = Newton(1/(1+S(AF_n)); seed BF~_n) in fp32;
            # G = M2^T AF ; H = G*BF_f (fp16) ; Y = column-sums via ones-matmul
            for c in range(N_CHUNK):
                psF = ppool.tile([128, FD], FP32, tag="ps")
                nc.tensor.matmul(out=psF[:], lhsT=w1h[:], rhs=af[c],
                                 start=True, stop=True)
                psG = ppool.tile([128, FD], FP32, tag="ps")
                nc.tensor.matmul(out=psG[:], lhsT=m2h[:], rhs=af[c],
                                 start=True, stop=True)
                bff = wpool.tile([128, FD], FP32, tag=f"bff{c}")
                newton1p(nc.vector, bff[:], psF[:], bf[c][:])
                h16 = wpool.tile([128, FD], FP16, tag=f"h{c}")
                nc.vector.tensor_mul(h16[:], psG[:], bff[:])
                psY = ppool.tile([128, FD], FP32, tag="ps")
                nc.tensor.matmul(out=psY[0:2, :], lhsT=onesb[:], rhs=h16[:],
                                 start=True, stop=True)
                ys = wpool.tile([128, FD], FP32, tag=f"ys{c}")
                nc.scalar.copy(ys[0:2, :], psY[0:2, :])
                nc.sync.dma_start(out=yout[2 * c:2 * c + 2, :], in_=ys[0:2, :])

    nc.finalize()
    return nc


def _get_module(repeat=1):
    key = f"nc{repeat}"
    if key not in _CACHE:
        _CACHE[key] = _build_module(repeat)
    return _CACHE[key]


def _blockdiag(a):
    out = np.zeros((128, 128), np.float16)
    out[0:64, 0:64] = a
    out[64:128, 64:128] = a
    return out


def kernel(AT, K_raw, BT_raw, W_raw, b_raw, _run_kw=None, _repeat=1):
    AT = np.asarray(AT, dtype=np.float32)
    K = np.clip(np.exp(np.asarray(K_raw, np.float32)), 0.0, 1000.0).astype(np.float32)
    BT = np.clip(np.exp(np.asarray(BT_raw, np.float32)), 0.0, 1000.0).astype(np.float32)
    Wc = np.clip(np.asarray(W_raw, np.float32), -10.0, 10.0).reshape(NA, NB)
    b0 = np.clip(np.asarray(b_raw, np.float32), -10.0, 10.0)[0]

    w1 = _blockdiag(K.astype(np.float16))                    # lhsT for S
    m2 = _blockdiag((K * Wc * BT[None, :]).astype(np.float16))
    w2k = [_blockdiag((OMEGAS[k] * (K * BT[None, :]).T).astype(np.float16))
           for k in range(N_ITERS)]
    wstack = np.ascontiguousarray(
        np.concatenate([w1, m2] + w2k, axis=1)).astype(np.float16)

    att = np.ascontiguousarray(AT.T).astype(np.float16)      # (64, 16384)

    in_maps = []
    for c in range(N_CORES):
        chunk = att[:, c * B_CORE:(c + 1) * B_CORE]          # (64, 2048)
        stacked = np.ascontiguousarray(
            np.concatenate([chunk[:, :B_CORE // 2], chunk[:, B_CORE // 2:]], axis=0))
        in_maps.append({"att": stacked, "wstack": wstack})

    nc = _get_module(_repeat)
    res = run_bass_kernel_spmd(nc, in_maps, core_ids=list(range(N_CORES)),
                               **(_run_kw or {}))
    out = np.empty((B,), np.float32)
    half = B_CORE // 2
    for c in range(N_CORES):
        y = res.results[c]["yout"]                           # (4, 512)
        base = c * B_CORE
        for ch in range(N_CHUNK):
            out[base + ch * FD: base + (ch + 1) * FD] = y[2 * ch]
            out[base + half + ch * FD: base + half + (ch + 1) * FD] = y[2 * ch + 1]
    if _run_kw is not None:
        _CACHE["last_result"] = res
    return out + b0


# revision 7
# speedup vs baseline: 1.2420x; 1.2420x over previous
"""Trainium2 Bass kernel for nn_CompetitiveNetwork (competitive-binding solve).

Math (per batch row b):
    K  = clip(exp(K_raw), 0, 1e3)   BT = clip(exp(BT_raw), 0, 1e3)
    fixed point:  BF' = 1/(1 + K^T AF);  AF = AT / (1 + (K*diag(BT)) BF')
    readout:      BF' = 1/(1 + K^T AF);  Y = AF^T (K*W*BT) BF' + b

v2 strategy (vs the 21-plain-iteration v1):
  - Successive over-relaxation on the AF-side reciprocal state R~:
      R~_k = (1-w_k) R~_{k-1} + w_k * Newton(1/(1+T_k); seed=R~_{k-1})
    folded into ONE custom DVE op per tile by pre-scaling the T-matmul
    weights with w_k (PSUM holds w_k*T, newton1p immediates s0=w_k,
    s1=1+w_k).  With a ramped w schedule 10 iterations reach ~1.6e-3
    rel err vs the 21-iteration reference (validated in numpy replica;
    harness gate is 2e-2).
  - Block-diagonal 128x128 fp16 weights: both batch-substreams in one
    matmul (PE cost = moving rows only), halving PE time vs quadrant
    pairs.
  - First ACT_R_ITERS iterations compute both reciprocals on ACT
    (exact, seedless) - the Newton basin needs |seed*(1+T)| < 2 which
    early iterates violate.
  - AT shipped as fp16 from host (halves input DMA, removes casts).

Sharding: pure data-parallel over batch (16384 -> 8 cores x 2048).
Layout: features on partitions (2 streams of 64 stacked -> 128), batch
on free dim; 2 column chunks of 512 per core.
"""

import numpy as np

import concourse.bacc as bacc
import concourse.mybir as mybir
from concourse.tile import TileContext
from concourse.bass_utils import run_bass_kernel_spmd


# --- custom DVE op: NEWTON1P_ANT (one 4-stage DVE instruction) ---
# out = (c1 - (in0 + c0) * in1) * in1
# With in0 = w*T (w pre-scaled into the matmul weights), c0 = w,
# c1 = 1+w, in1 = seed s:  out = (1-w)*s + w*(2-(1+T)s)s  — a Newton
# refinement of 1/(1+T) blended with SOR weight w, in one op.

import concourse.dve_ops as dve_ops
from concourse.dve_ops import DveOp
from concourse.dve_spec import Spec, Src0, Src1, C0, C1, lower


def _ref_newton1p(in0, in1, c0, c1, c2):
    return ((c1 - (in0.astype(np.float32) + c0) * in1) * in1).astype(np.float32)


def _make_op(shas):
    return DveOp(
        "NEWTON1P_ANT",
        Spec(
            body=(C1 - (Src0 + C0) * Src1) * Src1,
            reference=_ref_newton1p,
        ),
        subdim=False,
        uops_sha=shas,
    )


def register():
    for op in dve_ops.OPS:
        if op.name == "NEWTON1P_ANT":
            return op
    probe = _make_op({})
    opcode = dve_ops._CUSTOM_DVE_ROW_BASE + len(dve_ops.OPS)
    shas = {}
    for ver in ("v3", "v4"):
        try:
            from concourse.dve_uop import DveOpSpec
            res = DveOpSpec(name=probe.name, opcode=opcode,
                            uops=lower(probe.spec, ver=ver),
                            rd1_en=True)
            shas[ver] = res.sha(ver)
        except Exception as e:
            print(f"lower {ver} failed: {e}")
    op = _make_op(shas)
    dve_ops.OPS.append(op)
    dve_ops.CUSTOM_DVE_SPECS[op.name] = op.spec
    dve_ops._SUB_OPCODE_FOR_NAME[op.name] = (
        dve_ops._CUSTOM_DVE_ROW_BASE + len(dve_ops.OPS) - 1)
    return op


def newton1p(nc_vector, out, in0, in1, s0=1.0, s1=2.0):
    """out = (s1 - (in0 + s0) * in1) * in1 on the DVE."""
    op = register()
    return nc_vector._custom_dve(op, out=out, in0=in0, in1=in1,
                                 s0=s0, s1=s1, imm2=0.0)


B, NA, NB = 16384, 64, 64
N_CORES = 8
B_CORE = B // N_CORES          # 2048 batch rows per core
N_CHUNK = 2
FD = B_CORE // 2 // N_CHUNK    # 512

N_ITERS = 10
ACT_R_ITERS = 3                # seedless ACT reciprocals until Newton basin
# SOR schedule: identity while bootstrapping, then ramp (numpy-tuned)
OMEGAS = [1.0] * ACT_R_ITERS + [
    round(1.15 + (1.40 - 1.15) * i / (N_ITERS - ACT_R_ITERS - 1), 4)
    for i in range(N_ITERS - ACT_R_ITERS)
]

FP32 = mybir.dt.float32
FP16 = mybir.dt.float16

_CACHE = {}


def _act_recip(nc, out_ap, in_ap, bias=1.0, scale=1.0):
    """out = 1 / (scale*in + bias) on the ACT engine."""
    eng = nc.scalar
    ins = [eng.lower_ap(in_ap),
           mybir.ImmediateValue(dtype=FP32, value=bias),
           mybir.ImmediateValue(dtype=FP32, value=scale),
           mybir.ImmediateValue(dtype=FP32, value=0.0)]
    eng.add_instruction(mybir.InstActivation(
        name=nc.get_next_instruction_name(),
        func=mybir.ActivationFunctionType.Reciprocal,
        ins=ins, outs=[eng.lower_ap(out_ap)]))


def _build_module(repeat=1):
    register()
    nc = bacc.Bacc()
    # weight stack: [w1, m2, w2_0..w2_{n-1}] as 128x128 fp16 block-diagonals
    NW = 2 + N_ITERS
    wstack = nc.dram_tensor("wstack", (128, 128 * NW), FP16, kind="ExternalInput")
    att = nc.dram_tensor("att", (128, N_CHUNK * FD), FP16, kind="ExternalInput")
    yout = nc.dram_tensor("yout", (2 * N_CHUNK, FD), FP32, kind="ExternalOutput")

    with TileContext(nc) as tc, \
         tc.tile_pool(name="const", bufs=1) as cpool, \
         tc.tile_pool(name="state", bufs=3) as spool, \
         tc.tile_pool(name="work", bufs=2) as wpool, \
         tc.tile_pool(name="psum", bufs=8, space="PSUM") as ppool:

        wst = cpool.tile([128, 128 * NW], FP16, tag="wst")
        nc.sync.dma_start(out=wst[:], in_=wstack[:, :])
        w1h = wst[:, 0:128]
        m2h = wst[:, 128:256]
        w2h = [wst[:, 128 * (2 + k):128 * (3 + k)] for k in range(N_ITERS)]
        at16 = cpool.tile([128, N_CHUNK * FD], FP16, tag="at16")
        nc.sync.dma_start(out=at16[:], in_=att[:, :])
        # ones [128, 2]: col0 = ones on partitions 0:64, col1 on 64:128
        # (one matmul reduces both streams: out row 0 = stream0, row1 = stream1)
        onesb = cpool.tile([128, 2], FP16, tag="onesb")
        nc.vector.memset(onesb[:], 0.0)
        nc.vector.memset(onesb[0:64, 0:1], 1.0)
        nc.vector.memset(onesb[64:128, 1:2], 1.0)

        def at_sl(c):
            return at16[:, c * FD:(c + 1) * FD]

        for _rep in range(repeat):
            af = [at_sl(0), at_sl(1)]
            bf = [None] * N_CHUNK
            rr = [None] * N_CHUNK

            def emit_halfstep(c, h):
                k = h // 2
                w = OMEGAS[k]
                if h % 2 == 0:
                    # S = K^T AF ; BF' = 1/(1+S)
                    ps = ppool.tile([128, FD], FP32, tag="ps")
                    nc.tensor.matmul(out=ps[:], lhsT=w1h, rhs=af[c],
                                     start=True, stop=True)
                    bf_n = spool.tile([128, FD], FP16, tag=f"bf{c}")
                    _act_recip(nc, bf_n[:], ps[:])
                    bf[c] = bf_n
                else:
                    # PSUM = w*T (w baked into weights); R~ update
                    ps2 = ppool.tile([128, FD], FP32, tag="ps")
                    nc.tensor.matmul(out=ps2[:], lhsT=w2h[k], rhs=bf[c][:],
                                     start=True, stop=True)
                    r_n = spool.tile([128, FD], FP16, tag=f"r{c}")
                    if k < ACT_R_ITERS:
                        _act_recip(nc, r_n[:], ps2[:], bias=1.0, scale=1.0 / w)
                    else:
                        newton1p(nc.vector, r_n[:], ps2[:], rr[c][:],
                                 s0=w, s1=1.0 + w)
                    rr[c] = r_n
                    af_n = spool.tile([128, FD], FP16, tag=f"af{c}")
                    nc.vector.tensor_mul(af_n[:], at_sl(c), r_n[:])
                    af[c] = af_n

            # chunk 1 one half-step behind chunk 0 for steady pipelining
            H = 2 * N_ITERS
            for t in range(H + 1):
                if t < H:
                    emit_halfstep(0, t)
                if t >= 1:
                    emit_halfstep(1, t - 1)

            # readout: BF_f = Newton(1/(1+S(AF_n)); seed BF~_n) in fp32;
            # G = M2^T AF ; H = G*BF_f (fp16) ; Y = column-sums via ones-matmul
            for c in range(N_CHUNK):
                psF = ppool.tile([128, FD], FP32, tag="ps")
                nc.tensor.matmul(out=psF[:], lhsT=w1h, rhs=af[c],
                                 start=True, stop=True)
                psG = ppool.tile([128, FD], FP32, tag="ps")
                nc.tensor.matmul(out=psG[:], lhsT=m2h, rhs=af[c],
                                 start=True, stop=True)
                bff = wpool.tile([128, FD], FP32, tag=f"bff{c}")
                newton1p(nc.vector, bff[:], psF[:], bf[c][:])
                h16 = wpool.tile([128, FD], FP16, tag=f"h{c}")
                nc.vector.tensor_mul(h16[:], psG[:], bff[:])
                psY = ppool.tile([128, FD], FP32, tag="ps")
                nc.tensor.matmul(out=psY[0:2, :], lhsT=onesb[:], rhs=h16[:],
                                 start=True, stop=True)
                ys = wpool.tile([128, FD], FP32, tag=f"ys{c}")
                nc.vector.tensor_copy(ys[0:2, :], psY[0:2, :])
                nc.sync.dma_start(out=yout[2 * c:2 * c + 2, :], in_=ys[0:2, :])

    nc.finalize()
    return nc


def _get_module(repeat=1):
    key = f"nc{repeat}"
    if key not in _CACHE:
        _CACHE[key] = _build_module(repeat)
    return _CACHE[key]


def _blockdiag(a):
    out = np.zeros((128, 128), np.float16)
    out[0:64, 0:64] = a
    out[64:128, 64:128] = a
    return out


def kernel(AT, K_raw, BT_raw, W_raw, b_raw, _run_kw=None, _repeat=1):
    AT = np.asarray(AT, dtype=np.float32)
    K = np.clip(np.exp(np.asarray(K_raw, np.float32)), 0.0, 1000.0).astype(np.float32)
    BT = np.clip(np.exp(np.asarray(BT_raw, np.float32)), 0.0, 1000.0).astype(np.float32)
    Wc = np.clip(np.asarray(W_raw, np.float32), -10.0, 10.0).reshape(NA, NB)
    b0 = np.clip(np.asarray(b_raw, np.float32), -10.0, 10.0)[0]

    w1 = _blockdiag(K.astype(np.float16))                    # lhsT for S
    m2 = _blockdiag((K * Wc * BT[None, :]).astype(np.float16))
    w2k = [_blockdiag((OMEGAS[k] * (K * BT[None, :]).T).astype(np.float16))
           for k in range(N_ITERS)]
    wstack = np.ascontiguousarray(
        np.concatenate([w1, m2] + w2k, axis=1)).astype(np.float16)

    att = np.ascontiguousarray(AT.T).astype(np.float16)      # (64, 16384)

    in_maps = []
    for c in range(N_CORES):
        chunk = att[:, c * B_CORE:(c + 1) * B_CORE]          # (64, 2048)
        stacked = np.ascontiguousarray(
            np.concatenate([chunk[:, :B_CORE // 2], chunk[:, B_CORE // 2:]], axis=0))
        in_maps.append({"att": stacked, "wstack": wstack})

    nc = _get_module(_repeat)
    res = run_bass_kernel_spmd(nc, in_maps, core_ids=list(range(N_CORES)),
                               **(_run_kw or {}))
    out = np.empty((B,), np.float32)
    half = B_CORE // 2
    for c in range(N_CORES):
        y = res.results[c]["yout"]                           # (4, 512)
        base = c * B_CORE
        for ch in range(N_CHUNK):
            out[base + ch * FD: base + (ch + 1) * FD] = y[2 * ch]
            out[base + half + ch * FD: base + half + (ch + 1) * FD] = y[2 * ch + 1]
    if _run_kw is not None:
        _CACHE["last_result"] = res
    return out + b0


# revision 29
# speedup vs baseline: 1.4899x; 1.1996x over previous
"""Trainium2 Bass kernel for nn_CompetitiveNetwork (competitive-binding solve).

Math (per batch row b):
    K  = clip(exp(K_raw), 0, 1e3)   BT = clip(exp(BT_raw), 0, 1e3)
    fixed point:  BF' = 1/(1 + K^T AF);  AF = AT / (1 + (K*diag(BT)) BF')
    readout:      BF' = 1/(1 + K^T AF);  Y = AF^T (K*W*BT) BF' + b

v2 strategy (vs the 21-plain-iteration v1):
  - Successive over-relaxation on the AF-side reciprocal state R~:
      R~_k = (1-w_k) R~_{k-1} + w_k * Newton(1/(1+T_k); seed=R~_{k-1})
    folded into ONE custom DVE op per tile by pre-scaling the T-matmul
    weights with w_k (PSUM holds w_k*T, newton1p immediates s0=w_k,
    s1=1+w_k).  With a ramped w schedule 10 iterations reach ~1.6e-3
    rel err vs the 21-iteration reference (validated in numpy replica;
    harness gate is 2e-2).
  - Block-diagonal 128x128 fp16 weights: both batch-substreams in one
    matmul (PE cost = moving rows only), halving PE time vs quadrant
    pairs.
  - First ACT_R_ITERS iterations compute both reciprocals on ACT
    (exact, seedless) - the Newton basin needs |seed*(1+T)| < 2 which
    early iterates violate.
  - AT shipped as fp16 from host (halves input DMA, removes casts).

Sharding: pure data-parallel over batch (16384 -> 8 cores x 2048).
Layout: features on partitions (2 streams of 64 stacked -> 128), batch
on free dim; 2 column chunks of 512 per core.
"""

import numpy as np

import concourse.bacc as bacc
import concourse.mybir as mybir
from concourse.tile import TileContext
from concourse.bass_utils import run_bass_kernel_spmd


# --- custom DVE op: NEWTON1P_ANT (one 4-stage DVE instruction) ---
# out = (c1 - (in0 + c0) * in1) * in1
# With in0 = w*T (w pre-scaled into the matmul weights), c0 = w,
# c1 = 1+w, in1 = seed s:  out = (1-w)*s + w*(2-(1+T)s)s  — a Newton
# refinement of 1/(1+T) blended with SOR weight w, in one op.

import concourse.dve_ops as dve_ops
from concourse.dve_ops import DveOp
from concourse.dve_spec import Spec, Src0, Src1, C0, C1, lower


def _ref_newton1p(in0, in1, c0, c1, c2):
    return ((c1 - (in0.astype(np.float32) + c0) * in1) * in1).astype(np.float32)


def _make_op(shas):
    return DveOp(
        "NEWTON1P_ANT",
        Spec(
            body=(C1 - (Src0 + C0) * Src1) * Src1,
            reference=_ref_newton1p,
        ),
        subdim=False,
        uops_sha=shas,
    )


def register():
    for op in dve_ops.OPS:
        if op.name == "NEWTON1P_ANT":
            return op
    probe = _make_op({})
    opcode = dve_ops._CUSTOM_DVE_ROW_BASE + len(dve_ops.OPS)
    shas = {}
    for ver in ("v3", "v4"):
        try:
            from concourse.dve_uop import DveOpSpec
            res = DveOpSpec(name=probe.name, opcode=opcode,
                            uops=lower(probe.spec, ver=ver),
                            rd1_en=True)
            shas[ver] = res.sha(ver)
        except Exception as e:
            print(f"lower {ver} failed: {e}")
    op = _make_op(shas)
    dve_ops.OPS.append(op)
    dve_ops.CUSTOM_DVE_SPECS[op.name] = op.spec
    dve_ops._SUB_OPCODE_FOR_NAME[op.name] = (
        dve_ops._CUSTOM_DVE_ROW_BASE + len(dve_ops.OPS) - 1)
    return op


def newton1p(nc_vector, out, in0, in1, s0=1.0, s1=2.0):
    """out = (s1 - (in0 + s0) * in1) * in1 on the DVE."""
    op = register()
    return nc_vector._custom_dve(op, out=out, in0=in0, in1=in1,
                                 s0=s0, s1=s1, imm2=0.0)


B, NA, NB = 16384, 64, 64
N_CORES = 8
B_CORE = B // N_CORES          # 2048 batch rows per core
COLS = B_CORE // 2             # 1024 stacked columns per core
N_CHUNK = 3
_EDGES = [round(COLS * i / N_CHUNK) for i in range(N_CHUNK + 1)]
CHUNKS = list(zip(_EDGES[:-1], _EDGES[1:]))
FD = COLS                      # full width (DMA/readout use full range)

N_ITERS = 10
ACT_R_ITERS = 3                # seedless ACT reciprocals until Newton basin
# SOR schedule: identity while bootstrapping, then ramp (numpy-tuned)
OMEGAS = [1.0] * ACT_R_ITERS + [
    round(1.15 + (1.40 - 1.15) * i / (N_ITERS - ACT_R_ITERS - 1), 4)
    for i in range(N_ITERS - ACT_R_ITERS)
]

FP32 = mybir.dt.float32
FP16 = mybir.dt.float16

_CACHE = {}


def _act_recip(nc, out_ap, in_ap, bias=1.0, scale=1.0):
    """out = 1 / (scale*in + bias) on the ACT engine."""
    eng = nc.scalar
    ins = [eng.lower_ap(in_ap),
           mybir.ImmediateValue(dtype=FP32, value=bias),
           mybir.ImmediateValue(dtype=FP32, value=scale),
           mybir.ImmediateValue(dtype=FP32, value=0.0)]
    eng.add_instruction(mybir.InstActivation(
        name=nc.get_next_instruction_name(),
        func=mybir.ActivationFunctionType.Reciprocal,
        ins=ins, outs=[eng.lower_ap(out_ap)]))


def _build_module(repeat=1):
    register()
    nc = bacc.Bacc()
    # weight stack: [w1 | w2_0..w2_{n-1} | m2] as 128x128 fp16 block-diagonals
    NW = 2 + N_ITERS
    wstack = nc.dram_tensor("wstack", (128, 128 * NW), FP16, kind="ExternalInput")
    att = nc.dram_tensor("att", (128, COLS), FP16, kind="ExternalInput")
    yout = nc.dram_tensor("yout", (2, COLS), FP32, kind="ExternalOutput")

    with TileContext(nc) as tc, \
         tc.tile_pool(name="const", bufs=1) as cpool, \
         tc.tile_pool(name="state", bufs=3) as spool, \
         tc.tile_pool(name="work", bufs=2) as wpool, \
         tc.tile_pool(name="psum", bufs=2, space="PSUM") as ppool:

        # DMA order tuned for the head: att-c0 + the first weight blocks
        # arrive first (on separate DGE queues), bulk weights later.
        # wstack layout: [w1 | w2_0..w2_{n-1} | m2]
        at16 = cpool.tile([128, COLS], FP16, tag="at16")
        wst = cpool.tile([128, 128 * NW], FP16, tag="wst")
        nc.sync.dma_start(out=at16[:, 0:CHUNKS[0][1]], in_=att[:, 0:CHUNKS[0][1]])
        nc.gpsimd.dma_start(out=wst[:, 0:384], in_=wstack[:, 0:384])
        nc.sync.dma_start(out=at16[:, CHUNKS[0][1]:], in_=att[:, CHUNKS[0][1]:])
        nc.gpsimd.dma_start(out=wst[:, 384:], in_=wstack[:, 384:])
        w1h = wst[:, 0:128]
        w2h = [wst[:, 128 * (1 + k):128 * (2 + k)] for k in range(N_ITERS)]
        m2h = wst[:, 128 * (1 + N_ITERS):128 * (2 + N_ITERS)]
        # ones [128, 2]: col0 = ones on partitions 0:64, col1 on 64:128
        # (one matmul reduces both streams: out row 0 = stream0, row1 = stream1)
        onesb = cpool.tile([128, 2], FP16, tag="onesb")
        nc.vector.memset(onesb[:], 0.0)
        nc.vector.memset(onesb[0:64, 0:1], 1.0)
        nc.vector.memset(onesb[64:128, 1:2], 1.0)

        def at_sl(c):
            lo, hi = CHUNKS[c]
            return at16[:, lo:hi]

        for _rep in range(repeat):
            af = [at_sl(c) for c in range(N_CHUNK)]
            bf = [None] * N_CHUNK
            rr = [None] * N_CHUNK

            def emit_halfstep(c, h):
                k = h // 2
                w = OMEGAS[k]
                fdc = CHUNKS[c][1] - CHUNKS[c][0]
                if h % 2 == 0:
                    # S = K^T AF ; BF' = 1/(1+S)
                    ps = ppool.tile([128, fdc], FP32, tag=f"ps{c}")
                    nc.tensor.matmul(out=ps[:], lhsT=w1h, rhs=af[c],
                                     start=True, stop=True)
                    bf_n = spool.tile([128, fdc], FP16, tag=f"bf{c}")
                    _act_recip(nc, bf_n[:], ps[:])
                    bf[c] = bf_n
                else:
                    # PSUM = w*T (w baked into weights); R~ update
                    ps2 = ppool.tile([128, fdc], FP32, tag=f"ps{c}")
                    nc.tensor.matmul(out=ps2[:], lhsT=w2h[k], rhs=bf[c][:],
                                     start=True, stop=True)
                    r_n = spool.tile([128, fdc], FP16, tag=f"r{c}")
                    if k < ACT_R_ITERS:
                        _act_recip(nc, r_n[:], ps2[:], bias=1.0, scale=1.0 / w)
                    else:
                        newton1p(nc.vector, r_n[:], ps2[:], rr[c][:],
                                 s0=w, s1=1.0 + w)
                    rr[c] = r_n
                    af_n = spool.tile([128, fdc], FP16, tag=f"af{c}")
                    nc.vector.tensor_mul(af_n[:], at_sl(c), r_n[:])
                    af[c] = af_n

            # later chunks ripple one half-step behind for steady pipelining
            H = 2 * N_ITERS
            for t in range(H + N_CHUNK - 1):
                for c in range(N_CHUNK):
                    h = t - c
                    if 0 <= h < H:
                        emit_halfstep(c, h)

            # readout: BF_f = 1/(1+S(AF_n)) on ACT (exact, off the DVE);
            # G = M2^T AF ; H = G*BF_f (fp16) ; Y = column-sums via ones-matmul
            psF, psG, bff, h16 = [None] * N_CHUNK, [None] * N_CHUNK, \
                                 [None] * N_CHUNK, [None] * N_CHUNK
            ysw = wpool.tile([128, COLS], FP32, tag="ysw")
            for c in range(N_CHUNK):
                fdc = CHUNKS[c][1] - CHUNKS[c][0]
                psF[c] = ppool.tile([128, fdc], FP32, tag=f"ps{c}", name=f"psF{c}")
                nc.tensor.matmul(out=psF[c][:], lhsT=w1h, rhs=af[c],
                                 start=True, stop=True)
                psG[c] = ppool.tile([128, fdc], FP32, tag=f"ps{c}", name=f"psG{c}")
                nc.tensor.matmul(out=psG[c][:], lhsT=m2h, rhs=af[c],
                                 start=True, stop=True)
            for c in range(N_CHUNK):
                fdc = CHUNKS[c][1] - CHUNKS[c][0]
                bff[c] = wpool.tile([128, fdc], FP32, tag=f"bff{c}", name=f"bff{c}")
                _act_recip(nc, bff[c][:], psF[c][:])
                h16[c] = wpool.tile([128, fdc], FP16, tag=f"h{c}", name=f"h16{c}")
                nc.vector.tensor_mul(h16[c][:], psG[c][:], bff[c][:])
            for c in range(N_CHUNK):
                fdc = CHUNKS[c][1] - CHUNKS[c][0]
                psY = ppool.tile([128, fdc], FP32, tag=f"ps{c}", name=f"psY{c}")
                nc.tensor.matmul(out=psY[0:2, :], lhsT=onesb[:], rhs=h16[c][:],
                                 start=True, stop=True)
                nc.vector.tensor_copy(ysw[0:2, CHUNKS[c][0]:CHUNKS[c][1]],
                                      psY[0:2, :])
                if c == N_CHUNK - 1:
                    # earlier chunks' columns go out while c2's copy runs
                    nc.sync.dma_start(out=yout[0:2, 0:CHUNKS[c][0]],
                                      in_=ysw[0:2, 0:CHUNKS[c][0]])
            nc.sync.dma_start(out=yout[0:2, CHUNKS[-1][0]:],
                              in_=ysw[0:2, CHUNKS[-1][0]:])

    nc.finalize()
    return nc


def _get_module(repeat=1):
    key = f"nc{repeat}"
    if key not in _CACHE:
        _CACHE[key] = _build_module(repeat)
    return _CACHE[key]


def _blockdiag(a):
    out = np.zeros((128, 128), np.float16)
    out[0:64, 0:64] = a
    out[64:128, 64:128] = a
    return out


def kernel(AT, K_raw, BT_raw, W_raw, b_raw, _run_kw=None, _repeat=1):
    AT = np.asarray(AT, dtype=np.float32)
    K = np.clip(np.exp(np.asarray(K_raw, np.float32)), 0.0, 1000.0).astype(np.float32)
    BT = np.clip(np.exp(np.asarray(BT_raw, np.float32)), 0.0, 1000.0).astype(np.float32)
    Wc = np.clip(np.asarray(W_raw, np.float32), -10.0, 10.0).reshape(NA, NB)
    b0 = np.clip(np.asarray(b_raw, np.float32), -10.0, 10.0)[0]

    w1 = _blockdiag(K.astype(np.float16))                    # lhsT for S
    m2 = _blockdiag((K * Wc * BT[None, :]).astype(np.float16))
    w2k = [_blockdiag((OMEGAS[k] * (K * BT[None, :]).T).astype(np.float16))
           for k in range(N_ITERS)]
    wstack = np.ascontiguousarray(
        np.concatenate([w1] + w2k + [m2], axis=1)).astype(np.float16)

    att = np.ascontiguousarray(AT.T).astype(np.float16)      # (64, 16384)

    in_maps = []
    for c in range(N_CORES):
        chunk = att[:, c * B_CORE:(c + 1) * B_CORE]          # (64, 2048)
        stacked = np.ascontiguousarray(
            np.concatenate([chunk[:, :B_CORE // 2], chunk[:, B_CORE // 2:]], axis=0))
        in_maps.append({"att": stacked, "wstack": wstack})

    nc = _get_module(_repeat)
    res = run_bass_kernel_spmd(nc, in_maps, core_ids=list(range(N_CORES)),
                               **(_run_kw or {}))
    out = np.empty((B,), np.float32)
    half = B_CORE // 2
    for c in range(N_CORES):
        y = res.results[c]["yout"]                           # (2, 1024)
        base = c * B_CORE
        out[base: base + half] = y[0]
        out[base + half: base + B_CORE] = y[1]
    if _run_kw is not None:
        _CACHE["last_result"] = res
    return out + b0


# revision 33
# speedup vs baseline: 1.7470x; 1.1726x over previous
"""Trainium2 Bass kernel for nn_CompetitiveNetwork (competitive-binding solve).

Math (per batch row b):
    K  = clip(exp(K_raw), 0, 1e3)   BT = clip(exp(BT_raw), 0, 1e3)
    fixed point:  BF' = 1/(1 + K^T AF);  AF = AT / (1 + (K*diag(BT)) BF')
    readout:      BF' = 1/(1 + K^T AF);  Y = AF^T (K*W*BT) BF' + b

v2 strategy (vs the 21-plain-iteration v1):
  - Successive over-relaxation on the AF-side reciprocal state R~:
      R~_k = (1-w_k) R~_{k-1} + w_k * Newton(1/(1+T_k); seed=R~_{k-1})
    folded into ONE custom DVE op per tile by pre-scaling the T-matmul
    weights with w_k (PSUM holds w_k*T, newton1p immediates s0=w_k,
    s1=1+w_k).  With a ramped w schedule 10 iterations reach ~1.6e-3
    rel err vs the 21-iteration reference (validated in numpy replica;
    harness gate is 2e-2).
  - Block-diagonal 128x128 fp16 weights: both batch-substreams in one
    matmul (PE cost = moving rows only), halving PE time vs quadrant
    pairs.
  - First ACT_R_ITERS iterations compute both reciprocals on ACT
    (exact, seedless) - the Newton basin needs |seed*(1+T)| < 2 which
    early iterates violate.
  - AT shipped as fp16 from host (halves input DMA, removes casts).

Sharding: pure data-parallel over batch (16384 -> 8 cores x 2048).
Layout: features on partitions (2 streams of 64 stacked -> 128), batch
on free dim; 2 column chunks of 512 per core.
"""

import numpy as np

import concourse.bacc as bacc
import concourse.mybir as mybir
from concourse.tile import TileContext
from concourse.bass_utils import run_bass_kernel_spmd


# --- custom DVE op: NEWTON1P_ANT (one 4-stage DVE instruction) ---
# out = (c1 - (in0 + c0) * in1) * in1
# With in0 = w*T (w pre-scaled into the matmul weights), c0 = w,
# c1 = 1+w, in1 = seed s:  out = (1-w)*s + w*(2-(1+T)s)s  — a Newton
# refinement of 1/(1+T) blended with SOR weight w, in one op.

import concourse.dve_ops as dve_ops
from concourse.dve_ops import DveOp
from concourse.dve_spec import Spec, Src0, Src1, C0, C1, lower


def _ref_newton1p(in0, in1, c0, c1, c2):
    return ((c1 - (in0.astype(np.float32) + c0) * in1) * in1).astype(np.float32)


def _make_op(shas):
    return DveOp(
        "NEWTON1P_ANT",
        Spec(
            body=(C1 - (Src0 + C0) * Src1) * Src1,
            reference=_ref_newton1p,
        ),
        subdim=False,
        uops_sha=shas,
    )


def register():
    for op in dve_ops.OPS:
        if op.name == "NEWTON1P_ANT":
            return op
    probe = _make_op({})
    opcode = dve_ops._CUSTOM_DVE_ROW_BASE + len(dve_ops.OPS)
    shas = {}
    for ver in ("v3", "v4"):
        try:
            from concourse.dve_uop import DveOpSpec
            res = DveOpSpec(name=probe.name, opcode=opcode,
                            uops=lower(probe.spec, ver=ver),
                            rd1_en=True)
            shas[ver] = res.sha(ver)
        except Exception as e:
            print(f"lower {ver} failed: {e}")
    op = _make_op(shas)
    dve_ops.OPS.append(op)
    dve_ops.CUSTOM_DVE_SPECS[op.name] = op.spec
    dve_ops._SUB_OPCODE_FOR_NAME[op.name] = (
        dve_ops._CUSTOM_DVE_ROW_BASE + len(dve_ops.OPS) - 1)
    return op


def newton1p(nc_vector, out, in0, in1, s0=1.0, s1=2.0):
    """out = (s1 - (in0 + s0) * in1) * in1 on the DVE."""
    op = register()
    return nc_vector._custom_dve(op, out=out, in0=in0, in1=in1,
                                 s0=s0, s1=s1, imm2=0.0)


B, NA, NB = 16384, 64, 64
N_CORES = 8
B_CORE = B // N_CORES          # 2048 batch rows per core
COLS = B_CORE // 2             # 1024 stacked columns per core
N_CHUNK = 3
_EDGES = [round(COLS * i / N_CHUNK) for i in range(N_CHUNK + 1)]
CHUNKS = list(zip(_EDGES[:-1], _EDGES[1:]))
FD = COLS                      # full width (DMA/readout use full range)

N_ITERS = 8
ACT_R_ITERS = 3                # seedless ACT reciprocals until Newton basin
# Relaxation schedule (numpy random-search tuned on the replica; the
# alternating small/large pattern is Chebyshev-like): rel err 7.2e-4.
OMEGAS = [1.0] * ACT_R_ITERS + [1.062, 1.81, 1.053, 1.217, 1.8]
assert len(OMEGAS) == N_ITERS

FP32 = mybir.dt.float32
FP16 = mybir.dt.float16

_CACHE = {}


def _act_recip(nc, out_ap, in_ap, bias=1.0, scale=1.0,
               func=None):
    """out = func(scale*in + bias) on the ACT engine (default Reciprocal)."""
    eng = nc.scalar
    ins = [eng.lower_ap(in_ap),
           mybir.ImmediateValue(dtype=FP32, value=bias),
           mybir.ImmediateValue(dtype=FP32, value=scale),
           mybir.ImmediateValue(dtype=FP32, value=0.0)]
    eng.add_instruction(mybir.InstActivation(
        name=nc.get_next_instruction_name(),
        func=func or mybir.ActivationFunctionType.Reciprocal,
        ins=ins, outs=[eng.lower_ap(out_ap)]))


YSHIFT = 100.0  # relu(y+100)-100 == y for |y|<100: an ACT "copy" that stays
                # inside the reciprocal_and_small activation table


def _build_module(repeat=1):
    register()
    nc = bacc.Bacc()
    # weight stack: [w1 | w2_0..w2_{n-1} | m2] as 128x128 fp16 block-diagonals
    NW = 2 + N_ITERS
    wstack = nc.dram_tensor("wstack", (128, 128 * NW), FP16, kind="ExternalInput")
    att = nc.dram_tensor("att", (128, COLS), FP16, kind="ExternalInput")
    yout = nc.dram_tensor("yout", (2, COLS), FP32, kind="ExternalOutput")

    with TileContext(nc) as tc, \
         tc.tile_pool(name="const", bufs=1) as cpool, \
         tc.tile_pool(name="state", bufs=3) as spool, \
         tc.tile_pool(name="work", bufs=2) as wpool, \
         tc.tile_pool(name="psum", bufs=2, space="PSUM") as ppool:

        # DMA order tuned for the head: att-c0 + the first weight blocks
        # arrive first (on separate DGE queues), bulk weights later.
        # wstack layout: [w1 | w2_0..w2_{n-1} | m2]
        at16 = cpool.tile([128, COLS], FP16, tag="at16")
        wst = cpool.tile([128, 128 * NW], FP16, tag="wst")
        nc.sync.dma_start(out=at16[:, 0:CHUNKS[0][1]], in_=att[:, 0:CHUNKS[0][1]])
        nc.gpsimd.dma_start(out=wst[:, 0:384], in_=wstack[:, 0:384])
        nc.sync.dma_start(out=at16[:, CHUNKS[0][1]:], in_=att[:, CHUNKS[0][1]:])
        nc.gpsimd.dma_start(out=wst[:, 384:], in_=wstack[:, 384:])
        w1h = wst[:, 0:128]
        w2h = [wst[:, 128 * (1 + k):128 * (2 + k)] for k in range(N_ITERS)]
        m2h = wst[:, 128 * (1 + N_ITERS):128 * (2 + N_ITERS)]
        # ones [128, 2]: col0 = ones on partitions 0:64, col1 on 64:128
        # (one matmul reduces both streams: out row 0 = stream0, row1 = stream1)
        onesb = cpool.tile([128, 2], FP16, tag="onesb")
        nc.vector.memset(onesb[:], 0.0)
        nc.vector.memset(onesb[0:64, 0:1], 1.0)
        nc.vector.memset(onesb[64:128, 1:2], 1.0)

        def at_sl(c):
            lo, hi = CHUNKS[c]
            return at16[:, lo:hi]

        for _rep in range(repeat):
            af = [at_sl(c) for c in range(N_CHUNK)]
            bf = [None] * N_CHUNK
            rr = [None] * N_CHUNK

            def emit_halfstep(c, h):
                k = h // 2
                w = OMEGAS[k]
                fdc = CHUNKS[c][1] - CHUNKS[c][0]
                if h % 2 == 0:
                    # S = K^T AF ; BF' = 1/(1+S)
                    ps = ppool.tile([128, fdc], FP32, tag=f"ps{c}")
                    nc.tensor.matmul(out=ps[:], lhsT=w1h, rhs=af[c],
                                     start=True, stop=True)
                    bf_n = spool.tile([128, fdc], FP16, tag=f"bf{c}")
                    _act_recip(nc, bf_n[:], ps[:])
                    bf[c] = bf_n
                else:
                    # PSUM = w*T (w baked into weights); R~ update
                    ps2 = ppool.tile([128, fdc], FP32, tag=f"ps{c}")
                    nc.tensor.matmul(out=ps2[:], lhsT=w2h[k], rhs=bf[c][:],
                                     start=True, stop=True)
                    r_n = spool.tile([128, fdc], FP16, tag=f"r{c}")
                    if k < ACT_R_ITERS:
                        _act_recip(nc, r_n[:], ps2[:], bias=1.0, scale=1.0 / w)
                    else:
                        newton1p(nc.vector, r_n[:], ps2[:], rr[c][:],
                                 s0=w, s1=1.0 + w)
                    rr[c] = r_n
                    af_n = spool.tile([128, fdc], FP16, tag=f"af{c}")
                    nc.vector.tensor_mul(af_n[:], at_sl(c), r_n[:])
                    af[c] = af_n

            # later chunks ripple one half-step behind for steady pipelining
            H = 2 * N_ITERS
            for t in range(H + N_CHUNK - 1):
                for c in range(N_CHUNK):
                    h = t - c
                    if 0 <= h < H:
                        emit_halfstep(c, h)

            # readout: BF_f = 1/(1+S(AF_n)) on ACT (exact, off the DVE);
            # G = M2^T AF ; H = G*BF_f (fp16) ; Y = column-sums via ones-matmul
            psF, psG, bff, h16 = [None] * N_CHUNK, [None] * N_CHUNK, \
                                 [None] * N_CHUNK, [None] * N_CHUNK
            ysw = wpool.tile([128, COLS], FP32, tag="ysw")
            for c in range(N_CHUNK):
                fdc = CHUNKS[c][1] - CHUNKS[c][0]
                psF[c] = ppool.tile([128, fdc], FP32, tag=f"ps{c}", name=f"psF{c}")
                nc.tensor.matmul(out=psF[c][:], lhsT=w1h, rhs=af[c],
                                 start=True, stop=True)
                psG[c] = ppool.tile([128, fdc], FP32, tag=f"ps{c}", name=f"psG{c}")
                nc.tensor.matmul(out=psG[c][:], lhsT=m2h, rhs=af[c],
                                 start=True, stop=True)
            for c in range(N_CHUNK):
                fdc = CHUNKS[c][1] - CHUNKS[c][0]
                bff[c] = wpool.tile([128, fdc], FP32, tag=f"bff{c}", name=f"bff{c}")
                _act_recip(nc, bff[c][:], psF[c][:])
                h16[c] = wpool.tile([128, fdc], FP16, tag=f"h{c}", name=f"h16{c}")
                nc.vector.tensor_mul(h16[c][:], psG[c][:], bff[c][:])
            for c in range(N_CHUNK):
                fdc = CHUNKS[c][1] - CHUNKS[c][0]
                psY = ppool.tile([128, fdc], FP32, tag=f"ps{c}", name=f"psY{c}")
                nc.tensor.matmul(out=psY[0:2, :], lhsT=onesb[:], rhs=h16[c][:],
                                 start=True, stop=True)
                _act_recip(nc, ysw[0:2, CHUNKS[c][0]:CHUNKS[c][1]],
                           psY[0:2, :], bias=YSHIFT, scale=1.0,
                           func=mybir.ActivationFunctionType.Relu)
                if c == N_CHUNK - 1:
                    # earlier chunks' columns go out while c2's copy runs
                    nc.sync.dma_start(out=yout[0:2, 0:CHUNKS[c][0]],
                                      in_=ysw[0:2, 0:CHUNKS[c][0]])
            nc.sync.dma_start(out=yout[0:2, CHUNKS[-1][0]:],
                              in_=ysw[0:2, CHUNKS[-1][0]:])

    nc.finalize()
    return nc


def _get_module(repeat=1):
    key = f"nc{repeat}"
    if key not in _CACHE:
        _CACHE[key] = _build_module(repeat)
    return _CACHE[key]


def _blockdiag(a):
    out = np.zeros((128, 128), np.float16)
    out[0:64, 0:64] = a
    out[64:128, 64:128] = a
    return out


def kernel(AT, K_raw, BT_raw, W_raw, b_raw, _run_kw=None, _repeat=1):
    AT = np.asarray(AT, dtype=np.float32)
    K = np.clip(np.exp(np.asarray(K_raw, np.float32)), 0.0, 1000.0).astype(np.float32)
    BT = np.clip(np.exp(np.asarray(BT_raw, np.float32)), 0.0, 1000.0).astype(np.float32)
    Wc = np.clip(np.asarray(W_raw, np.float32), -10.0, 10.0).reshape(NA, NB)
    b0 = np.clip(np.asarray(b_raw, np.float32), -10.0, 10.0)[0]

    w1 = _blockdiag(K.astype(np.float16))                    # lhsT for S
    m2 = _blockdiag((K * Wc * BT[None, :]).astype(np.float16))
    w2k = [_blockdiag((OMEGAS[k] * (K * BT[None, :]).T).astype(np.float16))
           for k in range(N_ITERS)]
    wstack = np.ascontiguousarray(
        np.concatenate([w1] + w2k + [m2], axis=1)).astype(np.float16)

    att = np.ascontiguousarray(AT.T).astype(np.float16)      # (64, 16384)

    in_maps = []
    for c in range(N_CORES):
        chunk = att[:, c * B_CORE:(c + 1) * B_CORE]          # (64, 2048)
        stacked = np.ascontiguousarray(
            np.concatenate([chunk[:, :B_CORE // 2], chunk[:, B_CORE // 2:]], axis=0))
        in_maps.append({"att": stacked, "wstack": wstack})

    nc = _get_module(_repeat)
    res = run_bass_kernel_spmd(nc, in_maps, core_ids=list(range(N_CORES)),
                               **(_run_kw or {}))
    out = np.empty((B,), np.float32)
    half = B_CORE // 2
    for c in range(N_CORES):
        y = res.results[c]["yout"]                           # (2, 1024)
        base = c * B_CORE
        out[base: base + half] = y[0]
        out[base + half: base + B_CORE] = y[1]
    if _run_kw is not None:
        _CACHE["last_result"] = res
    return out + (b0 - YSHIFT)
